# revision 19
# baseline (speedup 1.0000x reference)
"""Trainium2 Bass kernel for nn_Decoder (dense transformer decoder).

Sharding: pure data-parallel over batch (32 -> 4 per core x 8 cores), no
collectives. Each core runs the full 4-layer decoder + vocab head for its
batch shard.

Device layout: feature-major activations [feat_part, b*T + t free].
Attention computed as scores^T [s, t] so no on-device transposes are needed;
softmax normalization is deferred (exp -> ones-matmul denominator over
partitions -> K=1 broadcast matmul -> multiply).

Host-side (free) prep: embedding gather + pos table, mask->bias conversion,
folding of the no-nonlinearity linear pairs in the h/v streams into the
fuse projection, per-batch context rows, final vocab bias, bf16 weight cast.
"""

import math

import numpy as np
import ml_dtypes

import concourse.bass as bass
import concourse.mybir as mybir
import concourse.tile as tile
from concourse.bass_utils import run_bass_kernel_spmd
from concourse.vector_clock import ScopedClock

# ---------------------------------------------------------------- constants
P = 128
L = 4
NCORES = 8
B = 32
BL = B // NCORES          # 4 local batches per core
T = 128
NT = BL * T               # 512 token columns per core
S = 160                   # encoder length
EC = BL * S               # 640 encoder columns per core
D = 512
H = 8
DK = 64
DF = 2048
DC = 1024
V = 32000
LN_EPS = 1e-5
NEG = -3.0e29             # large-negative mask bias (exp -> exact 0 in f32)

BF = mybir.dt.bfloat16
F32 = mybir.dt.float32
AF = mybir.ActivationFunctionType
ALU = mybir.AluOpType
bf16 = ml_dtypes.bfloat16

# vocab slicing for the head: 64 slices of 500 = 32000
VSLICE = 500
NVS = V // VSLICE         # 64
HALF_VS = NVS // 2        # 32 slices per oW2 slab


# ------------------------------------------------- tile drain-limit patch
def _patched_drain_and_barrier(self, tick_clock, wait_clock):
    # walrus CoreV3 rejects a Drain carrying >2 sem waits; park the waits on
    # dedicated NOPs in front of it instead.
    nc = self.nc
    nops = [nc.sync.nop(nofuse=True, hint=f"drain_wait_{i}") for i in range(48)]
    drain_inst = nc.sync.drain()
    wait_clock.add_sem_waits(
        drain_inst.ins, ScopedClock({None: tick_clock.global_clock})
    )
    si = drain_inst.ins.sync_info
    waits = list(si.on_wait) if si and si.on_wait else []
    if len(waits) > 1:
        keep, move = waits[:1], waits[1:]
        assert len(move) <= len(nops), f"too many drain waits: {len(waits)}"
        si.on_wait = keep
        for w, nop in zip(move, nops):
            nsi = nop.ins.sync_info
            if nsi is None:
                nop.ins.sync_info = mybir.SyncInfo(on_wait=[w], on_update=[])
            else:
                nsi.on_wait = (list(nsi.on_wait) if nsi.on_wait else []) + [w]
    nc.all_engine_barrier()
    assert self.sems is not None
    popped = nc._tile_sem_poison_stack.pop()
    assert popped is self._sem_poison
    nc.clear_and_free_semaphores(list(self.sems.allocated().values()))
    nc.all_engine_barrier()


tile.TileContext._drain_and_barrier = _patched_drain_and_barrier

_MAXW = 1  # this walrus codegen accepts only one sem wait per instruction


def _cap_sync_waits(nc):
    """Move excess sem waits (beyond _MAXW) onto inserted same-engine NoOps
    directly preceding the instruction — sound because per-engine program
    order guarantees the NoOp's waits complete before the instruction runs."""
    uid = [0]
    for f in nc.m.functions:
        for bb in f.blocks:
            il = bb.instructions
            out = []
            changed = False
            for inst in il:
                si = inst.sync_info
                waits = list(si.on_wait) if si and si.on_wait else []
                if len(waits) > _MAXW and inst.engine != mybir.EngineType.Unassigned:
                    keep, move = waits[:_MAXW], waits[_MAXW:]
                    si.on_wait = keep
                    for i in range(0, len(move), _MAXW):
                        uid[0] += 1
                        nop = mybir.InstNoOp(name=f"I-capw-{uid[0]}", ins=[],
                                             outs=[])
                        nop.engine = inst.engine
                        nop.sync_info = mybir.SyncInfo(
                            on_wait=move[i:i + _MAXW], on_update=[])
                        nc.register_instruction(nop, overwrite=True)
                        out.append(nop)
                    changed = True
                out.append(inst)
            if changed:
                bb.instructions = out


# ---------------------------------------------------------------- builder
def build_nc():
    nc = bass.Bass()

    def din(name, shape, dt=BF):
        return nc.declare_dram_parameter(name, list(shape), dt, isOutput=False)

    d = {}
    d["x0T"] = din("x0T", [4, P, NT])
    d["encT"] = din("encT", [4, P, EC])
    for nm in ("Wq_s", "Wk_s", "Wv_s", "Wq_e", "Wk_e", "Wv_e"):
        d[nm] = din(nm, [L, 4, P, 512])
    for nm in ("bq_s", "bk_s", "bv_s", "bq_e", "bk_e", "bv_e"):
        d[nm] = din(nm, [L, P, 4], F32)
    d["M_hv"] = din("M_hv", [L, 4, P, 512])
    d["c_hv"] = din("c_hv", [L, P, 16], F32)
    d["C_hv"] = din("C_hv", [L, 4, P, DC])
    d["Pb"] = din("Pb", [L, P, 8], F32)
    d["fW1"] = din("fW1", [L, 8, P, 512])
    d["fb1"] = din("fb1", [L, P, 4], F32)
    d["fnnW0"] = din("fnnW0", [L, 4, P, DF])
    d["fnnb0"] = din("fnnb0", [L, P, 16], F32)
    d["fnnW1"] = din("fnnW1", [L, 16, P, 512])
    d["fnnb1"] = din("fnnb1", [L, P, 4], F32)
    d["lng"] = din("lng", [L, P, 4], F32)
    d["lnb"] = din("lnb", [L, P, 4], F32)
    d["oW0"] = din("oW0", [4, P, 256])
    d["ob0"] = din("ob0", [P, 2], F32)
    d["oW1"] = din("oW1", [2, P, P])
    d["ob1"] = din("ob1", [P, 1], F32)
    d["oW2"] = din("oW2", [P, V])
    d["caus01"] = din("caus01", [P, P])                 # bf16 0/1 (s<=t)
    d["smask"] = din("smask", [P, BL], F32)             # self key-mask bias
    d["emask0"] = din("emask0", [P, BL], F32)           # enc key-mask bias s<128
    d["emask1"] = din("emask1", [32, BL], F32)          # enc key-mask bias s>=128
    out = nc.declare_dram_parameter("out", [NT, V], F32, isOutput=True)

    with tile.TileContext(nc) as tc:
        _emit(nc, tc, d, out)
    _cap_sync_waits(nc)
    return nc


def _emit(nc, tc, d, out):
    import contextlib

    ctx = contextlib.ExitStack()
    with ctx:
        pw = ctx.enter_context(tc.tile_pool(name="wts", bufs=1))
        px = ctx.enter_context(tc.tile_pool(name="xgen", bufs=2))
        pa = ctx.enter_context(tc.tile_pool(name="acts", bufs=1))
        pb2 = ctx.enter_context(tc.tile_pool(name="acts2", bufs=2))
        sm = ctx.enter_context(tc.tile_pool(name="small", bufs=2))
        ph = ctx.enter_context(tc.tile_pool(name="headw", bufs=1))
        pj = ctx.enter_context(tc.tile_pool(name="psA", bufs=2, space="PSUM"))
        scp = ctx.enter_context(tc.tile_pool(name="psB", bufs=2, space="PSUM"))
        opp = ctx.enter_context(tc.tile_pool(name="psC", bufs=2, space="PSUM"))
        lbp = ctx.enter_context(tc.tile_pool(name="psD", bufs=2, space="PSUM"))

        # ---------------- constants / global tiles
        ones128 = sm.tile([P, 1], BF, tag="c_ones128")
        nc.any.memset(ones128[:], 1.0)
        ones1f = sm.tile([1, P], F32, tag="c_ones1f")
        nc.any.memset(ones1f[:], 1.0)
        caus = sm.tile([P, P], BF, tag="c_caus")
        nc.sync.dma_start(caus[:], d["caus01"][:])
        smask = sm.tile([P, BL], F32, tag="c_smask")
        nc.sync.dma_start(smask[:], d["smask"][:])
        emask0 = sm.tile([P, BL], F32, tag="c_emask0")
        nc.sync.dma_start(emask0[:], d["emask0"][:])
        emask1 = sm.tile([32, BL], F32, tag="c_emask1")
        nc.sync.dma_start(emask1[:], d["emask1"][:])

        enc = []
        for ft in range(4):
            e = pa.tile([P, EC], BF, tag=f"enc{ft}")
            nc.sync.dma_start(e[:], d["encT"][ft])
            enc.append(e)
        xg = []
        for ft in range(4):
            x = px.tile([P, NT], BF, tag=f"xg{ft}")
            nc.sync.dma_start(x[:], d["x0T"][ft])
            xg.append(x)

        # -------------- helpers -------------------------------------------
        def load_w(name, l, nchunks, ncols, tag=None):
            """DMA a [L, nchunks, P, ncols] bf16 weight's layer l into SBUF."""
            t = pw.tile([P, nchunks, ncols], BF, tag=tag or name)
            for kc in range(nchunks):
                nc.sync.dma_start(t[:, kc, :], d[name][l, kc])
            return t

        def load_b(name, l, ncols, tag=None):
            # per-layer tag: tiny tiles, avoids cross-layer slot-wait DMAs
            t = pw.tile([P, ncols], F32, tag=f"{tag or name}{l}",
                        name=f"{tag or name}{l}")
            nc.sync.dma_start(t[:], d[name][l])
            return t

        def proj_feature_major(w_sb, rhs_tiles, nmt, bias_sb, out_tag, relu=False,
                               ncols=NT, pool=pa):
            """out[mt][P, ncols] (bf16) = w^T @ rhs (+bias), feature-major."""
            outs = []
            fn = AF.Relu if relu else AF.Identity
            nkc = w_sb.shape[1]
            for mt in range(nmt):
                o = pool.tile([P, ncols], BF, tag=f"{out_tag}{mt}")
                for cs in range(0, ncols, 512):
                    cw = min(512, ncols - cs)
                    ps = pj.tile([P, 512], F32, tag="pj")
                    for kc in range(nkc):
                        nc.tensor.matmul(
                            ps[:, :cw],
                            w_sb[:, kc, mt * P:(mt + 1) * P],
                            rhs_tiles[kc][:, cs:cs + cw],
                            start=(kc == 0), stop=(kc == nkc - 1),
                        )
                    nc.scalar.activation(o[:, cs:cs + cw], ps[:, :cw], fn,
                                         bias=bias_sb[:, mt:mt + 1])
                outs.append(o)
            return outs

        def layer_norm(l, x_tiles, deltas, bias_sb, lng_sb, lnb_sb, mk_delta=None):
            """new_x = LN(x + delta + bias_percol).  deltas: list of 4 APs
            (sbuf or psum) or None with mk_delta(ft) producing the psum."""
            r_tiles = []
            sr_ps = scp.tile([1, NT], F32, tag="sc")
            s2_ps = scp.tile([1, NT], F32, tag="sc")
            for ft in range(4):
                dl = deltas[ft] if deltas is not None else mk_delta(ft)
                tmp = pa.tile([P, NT], BF, tag="ln_tmp")
                nc.vector.tensor_tensor(tmp[:], x_tiles[ft][:], dl, ALU.add)
                r = pa.tile([P, NT], BF, tag=f"ln_r{ft}")
                nc.scalar.activation(r[:], tmp[:], AF.Identity,
                                     bias=bias_sb[:, ft:ft + 1])
                r_tiles.append(r)
                nc.tensor.matmul(sr_ps[:], ones128[:], r[:],
                                 start=(ft == 0), stop=(ft == 3))
                r2 = pa.tile([P, NT], BF, tag=f"ln_r2_{ft}", name=f"r2_{ft}")
                nc.scalar.square(r2[:], r[:])
                nc.tensor.matmul(s2_ps[:], ones128[:], r2[:],
                                 start=(ft == 0), stop=(ft == 3))
            # packed stats rows: 4 regions of NT, reused as values die
            rows = sm.tile([1, 4 * NT], F32, tag="lnrow")
            mu = rows[:, 0 * NT:1 * NT]
            m2e = rows[:, 1 * NT:2 * NT]
            msq = rows[:, 2 * NT:3 * NT]
            veps = rows[:, 3 * NT:4 * NT]
            sq = rows[:, 1 * NT:2 * NT]     # reuses m2e (dead after veps)
            rstd = rows[:, 3 * NT:4 * NT]   # reuses veps (dead after sq)
            mrs = rows[:, 2 * NT:3 * NT]    # reuses msq (dead after veps)
            nc.vector.tensor_scalar_mul(mu, sr_ps[:], 1.0 / D)
            nc.vector.tensor_scalar(m2e, s2_ps[:], 1.0 / D, LN_EPS,
                                    ALU.mult, ALU.add)
            nc.scalar.square(msq, mu)
            nc.vector.tensor_tensor(veps, m2e, msq, ALU.subtract)
            nc.scalar.sqrt(sq, veps)
            nc.vector.reciprocal(rstd, sq)
            nc.vector.tensor_tensor(mrs, mu, rstd, ALU.mult)
            R_ps = lbp.tile([P, NT], F32, tag="lb")
            nc.tensor.matmul(R_ps[:], ones1f[:, :P], rstd, start=True, stop=True)
            M_ps = lbp.tile([P, NT], F32, tag="lb")
            nc.tensor.matmul(M_ps[:], ones1f[:, :P], mrs, start=True, stop=True)
            outs = []
            for ft in range(4):
                u = sm.tile([P, NT], BF, tag="ln_u")
                nc.vector.tensor_tensor(u[:], r_tiles[ft][:], M_ps[:],
                                        ALU.subtract)
                w = sm.tile([P, NT], BF, tag="ln_w")
                nc.vector.tensor_tensor(w[:], u[:], R_ps[:], ALU.mult)
                xo = px.tile([P, NT], BF, tag=f"xg{ft}")
                nc.vector.tensor_scalar(xo[:], w[:], lng_sb[:, ft:ft + 1],
                                        lnb_sb[:, ft:ft + 1], ALU.mult, ALU.add)
                outs.append(xo)
            return outs

        def attention(qT, kT, v0, v1, e0_tag, e1_tag, slen, mask0, mask1,
                      out_tag, ctx_stride):
            """Generic attention. qT/kT: 4 feature-major tiles ([P, NT] /
            [P, BL*ctx_stride]); v0/v1: per-b value tiles [s-chunk, 512].
            Returns 4 out^T tiles [P, NT] (unnormalized bias deferred)."""
            oT = [pb2.tile([P, NT], BF, tag=f"{out_tag}{j}",
                           name=f"{out_tag}{j}") for j in range(4)]
            s0 = min(slen, P)
            s1 = slen - s0
            for b in range(BL):
                bq = slice(b * T, (b + 1) * T)
                c0 = slice(b * ctx_stride, b * ctx_stride + s0)
                c1 = slice(b * ctx_stride + s0, b * ctx_stride + slen)
                e0 = pb2.tile([P, H * T], BF, tag=e0_tag)
                e1 = None
                if s1:
                    e1 = pb2.tile([s1, H * T], BF, tag=e1_tag)
                for h in range(H):
                    ht, hr = h // 2, (h % 2) * 64
                    hs = slice(hr, hr + 64)
                    ecol = slice(h * T, (h + 1) * T)
                    sc_ps = scp.tile([P, T], F32, tag="sc")
                    nc.tensor.matmul(sc_ps[:s0, :], kT[ht][hs, c0],
                                     qT[ht][hs, bq], start=True, stop=True)
                    if mask1 is None:
                        # self-attn: exp then exact-zero causal multiply
                        et = sm.tile([P, T], BF, tag="etmp")
                        nc.scalar.activation(et[:], sc_ps[:], AF.Exp,
                                             scale=0.125, bias=mask0[:, b:b + 1])
                        nc.vector.tensor_tensor(e0[:, ecol], et[:], caus[:],
                                                ALU.mult)
                    else:
                        nc.scalar.activation(e0[:, ecol], sc_ps[:s0, :], AF.Exp,
                                             scale=0.125, bias=mask0[:, b:b + 1])
                    if s1:
                        sc1 = scp.tile([s1, T], F32, tag="sc")
                        nc.tensor.matmul(sc1[:], kT[ht][hs, c1], qT[ht][hs, bq],
                                         start=True, stop=True)
                        nc.scalar.activation(e1[:, ecol], sc1[:], AF.Exp,
                                             scale=0.125, bias=mask1[:, b:b + 1])
                rec = sm.tile([1, H * T], F32, tag="rec")
                for half in range(2):
                    hcol = slice(half * 512, (half + 1) * 512)
                    dps = scp.tile([1, 512], F32, tag="sc")
                    nc.tensor.matmul(dps[:], ones128[:], e0[:, hcol],
                                     start=True, stop=(s1 == 0))
                    if s1:
                        nc.tensor.matmul(dps[:], ones128[:s1, :], e1[:, hcol],
                                         start=False, stop=True)
                    nc.vector.reciprocal(rec[:, hcol], dps[:])
                for j in range(4):
                    op_ps = opp.tile([P, T], F32, tag="op")
                    bc_ps = opp.tile([P, T], F32, tag="op")
                    for i in range(2):
                        h = 2 * j + i
                        rsl = slice(i * 64, (i + 1) * 64)
                        # v columns for head h: h*64..h*64+64 within [*,512]
                        vcol = slice(h * 64, (h + 1) * 64)
                        ecol = slice(h * T, (h + 1) * T)
                        nc.tensor.matmul(op_ps[rsl, :], v0[b][:, vcol],
                                         e0[:, ecol], start=True, stop=(s1 == 0))
                        if s1:
                            nc.tensor.matmul(op_ps[rsl, :], v1[b][:, vcol],
                                             e1[:, ecol], start=False, stop=True)
                        nc.tensor.matmul(bc_ps[rsl, :], ones1f[:, :64],
                                         rec[:, ecol], start=True, stop=True)
                    bc_sb = sm.tile([P, T], F32, tag="bcsb")
                    nc.scalar.copy(bc_sb[:], bc_ps[:])
                    nc.vector.tensor_tensor(oT[j][:, bq], op_ps[:], bc_sb[:],
                                            ALU.mult)
            return oT

        # ---------------- the 4 layers ------------------------------------
        for l in range(L):
            lng_sb = load_b("lng", l, 4)
            lnb_sb = load_b("lnb", l, 4)

            # -- self attention
            Wq = load_w("Wq_s", l, 4, 512)
            Wk = load_w("Wk_s", l, 4, 512)
            Wv = load_w("Wv_s", l, 4, 512)
            bq_sb = load_b("bq_s", l, 4)
            bk_sb = load_b("bk_s", l, 4)
            bv_sb = load_b("bv_s", l, 4)
            qT = proj_feature_major(Wq, xg, 4, bq_sb, "qT")
            kT = proj_feature_major(Wk, xg, 4, bk_sb, "kT")
            vs = []
            for b in range(BL):
                ps = pj.tile([P, 512], F32, tag="pj")
                for kc in range(4):
                    nc.tensor.matmul(ps[:], xg[kc][:, b * T:(b + 1) * T],
                                     Wv[:, kc, :], start=(kc == 0), stop=(kc == 3))
                vb = pb2.tile([P, 512], BF, tag="v0")
                nc.any.tensor_copy(out=vb[:], in_=ps[:])
                vs.append(vb)
            saT = attention(qT, kT, vs, None, "e0", "e1", T, smask, None,
                            "aT", T)
            x1 = layer_norm(l, xg, [t[:] for t in saT], bv_sb, lng_sb, lnb_sb)

            # -- cross attention
            Wqe = load_w("Wq_e", l, 4, 512)
            Wke = load_w("Wk_e", l, 4, 512)
            Wve = load_w("Wv_e", l, 4, 512)
            bqe_sb = load_b("bq_e", l, 4)
            bke_sb = load_b("bk_e", l, 4)
            bve_sb = load_b("bv_e", l, 4)
            qTe = proj_feature_major(Wqe, x1, 4, bqe_sb, "qT")
            kTe = proj_feature_major(Wke, enc, 4, bke_sb, "kT", ncols=EC)
            ve0, ve1 = [], []
            for b in range(BL):
                ps = pj.tile([P, 512], F32, tag="pj")
                for kc in range(4):
                    nc.tensor.matmul(ps[:], enc[kc][:, b * S:b * S + P],
                                     Wve[:, kc, :], start=(kc == 0), stop=(kc == 3))
                vb = pb2.tile([P, 512], BF, tag="v0")
                nc.any.tensor_copy(out=vb[:], in_=ps[:])
                ve0.append(vb)
                ps1 = pj.tile([P, 512], F32, tag="pj")
                for kc in range(4):
                    nc.tensor.matmul(ps1[:32, :], enc[kc][:, b * S + P:(b + 1) * S],
                                     Wve[:, kc, :], start=(kc == 0), stop=(kc == 3))
                vb1 = pb2.tile([32, 512], BF, tag="ve1")
                nc.any.tensor_copy(out=vb1[:], in_=ps1[:32, :])
                ve1.append(vb1)
            eaT = attention(qTe, kTe, ve0, ve1, "e0", "ec1", S, emask0, emask1,
                            "aT", S)
            x2 = layer_norm(l, x1, [t[:] for t in eaT], bve_sb, lng_sb, lnb_sb)

            # -- h/v streams (folded) + fuse
            Mhv = load_w("M_hv", l, 4, 512)
            chv_sb = load_b("c_hv", l, 16)
            R = []
            for mt in range(4):
                ps = pj.tile([P, 512], F32, tag="pj")
                for kc in range(4):
                    nc.tensor.matmul(ps[:], Mhv[:, kc, mt * P:(mt + 1) * P],
                                     x2[kc][:], start=(kc == 0), stop=(kc == 3))
                r = pa.tile([P, NT], BF, tag=f"ln_r{mt}")
                for b in range(BL):
                    bq = slice(b * T, (b + 1) * T)
                    nc.scalar.activation(r[:, bq], ps[:, bq], AF.Relu,
                                         bias=chv_sb[:, mt * 4 + b:mt * 4 + b + 1])
                R.append(r)
            Chv = load_w("C_hv", l, 4, DC)
            Pb_sb = load_b("Pb", l, 8)
            F = []
            for mt in range(8):
                ps = pj.tile([P, 512], F32, tag="pj")
                for kc in range(4):
                    nc.tensor.matmul(ps[:], Chv[:, kc, mt * P:(mt + 1) * P],
                                     R[kc][:], start=(kc == 0), stop=(kc == 3))
                f = pa.tile([P, NT], BF, tag=f"F{mt}")
                nc.scalar.activation(f[:], ps[:], AF.Relu,
                                     bias=Pb_sb[:, mt:mt + 1])
                F.append(f)
            fW1 = load_w("fW1", l, 8, 512)
            fb1_sb = load_b("fb1", l, 4)

            def mk_fuse(ft):
                ps = pj.tile([P, 512], F32, tag="pj")
                for kc in range(8):
                    nc.tensor.matmul(ps[:], fW1[:, kc, ft * P:(ft + 1) * P],
                                     F[kc][:], start=(kc == 0), stop=(kc == 7))
                return ps[:]

            x3 = layer_norm(l, x2, None, fb1_sb, lng_sb, lnb_sb, mk_delta=mk_fuse)

            # -- FFN
            fW0 = load_w("fnnW0", l, 4, DF)
            fb0_sb = load_b("fnnb0", l, 16)
            G = []
            for mt in range(16):
                ps = pj.tile([P, 512], F32, tag="pj")
                for kc in range(4):
                    nc.tensor.matmul(ps[:], fW0[:, kc, mt * P:(mt + 1) * P],
                                     x3[kc][:], start=(kc == 0), stop=(kc == 3))
                g = pa.tile([P, NT], BF, tag=f"G{mt}")
                nc.scalar.activation(g[:], ps[:], AF.Relu,
                                     bias=fb0_sb[:, mt:mt + 1])
                G.append(g)
            gW1 = load_w("fnnW1", l, 16, 512)
            gb1_sb = load_b("fnnb1", l, 4)

            def mk_ffn(ft):
                ps = pj.tile([P, 512], F32, tag="pj")
                for kc in range(16):
                    nc.tensor.matmul(ps[:], gW1[:, kc, ft * P:(ft + 1) * P],
                                     G[kc][:], start=(kc == 0), stop=(kc == 15))
                return ps[:]

            xg = layer_norm(l, x3, None, gb1_sb, lng_sb, lnb_sb, mk_delta=mk_ffn)

        # ---------------- output head ------------------------------------
        oW0 = pw.tile([P, 4, 256], BF, tag="oW0")
        for kc in range(4):
            nc.sync.dma_start(oW0[:, kc, :], d["oW0"][kc])
        ob0_sb = pw.tile([P, 2], F32, tag="ob0")
        nc.sync.dma_start(ob0_sb[:], d["ob0"][:])
        h1 = proj_feature_major(oW0, xg, 2, ob0_sb, "qT", relu=True)
        oW1 = pw.tile([P, 2, P], BF, tag="oW1")
        for kc in range(2):
            nc.sync.dma_start(oW1[:, kc, :], d["oW1"][kc])
        ob1_sb = pw.tile([P, 1], F32, tag="ob1")
        nc.sync.dma_start(ob1_sb[:], d["ob1"][:])
        h2 = proj_feature_major(oW1, h1, 1, ob1_sb, "kT", relu=True)[0]

        SLAB_VS = 8                      # vocab slices per oW2 slab (8*500=4000)
        for sl in range(NVS // SLAB_VS):
            slab = ph.tile([P, SLAB_VS * VSLICE], BF, tag="oW2")
            base = sl * SLAB_VS * VSLICE
            nc.sync.dma_start(slab[:], d["oW2"][:, base:base + SLAB_VS * VSLICE])
            for tt_ in range(4):
                lhsT_tok = h2[:, tt_ * T:(tt_ + 1) * T]
                for v_ in range(SLAB_VS):
                    ps = pj.tile([P, 512], F32, tag="pj")
                    nc.tensor.matmul(ps[:, :VSLICE], lhsT_tok,
                                     slab[:, v_ * VSLICE:(v_ + 1) * VSLICE],
                                     start=True, stop=True)
                    og = sm.tile([P, 512], F32, tag="og")
                    nc.any.tensor_copy(out=og[:, :VSLICE], in_=ps[:, :VSLICE])
                    vbase = base + v_ * VSLICE
                    nc.sync.dma_start(
                        out[tt_ * T:(tt_ + 1) * T, vbase:vbase + VSLICE],
                        og[:, :VSLICE])


# ------------------------------------------------------------- host prep
def _pos_table(max_len, dm):
    pe = np.array([[p / np.power(10000, 2 * (k // 2) / dm) for k in range(dm)]
                   if p != 0 else np.zeros(dm) for p in range(max_len)])
    pe[1:, 0::2] = np.sin(pe[1:, 0::2])
    pe[1:, 1::2] = np.cos(pe[1:, 1::2])
    return pe.astype(np.float32)


def _chunk_w(w, ncols):
    """[K, M] f32 -> [K//128, 128, M] bf16."""
    K = w.shape[0]
    return np.ascontiguousarray(
        w.reshape(K // P, P, ncols).astype(bf16))


def _bias_cols(b):
    """[N] -> [128, N//128] f32 (partition-major per 128-tile)."""
    n = b.shape[0]
    return np.ascontiguousarray(b.reshape(n // P, P).T.astype(np.float32))


def prep_inputs(input_batch, encoder_out, history_context, visual_context,
                input_mask, enc_mask, params):
    input_batch = np.asarray(input_batch)
    encoder_out = np.asarray(encoder_out, np.float32)
    history_context = np.asarray(history_context, np.float32)
    visual_context = np.asarray(visual_context, np.float32)
    input_mask = np.asarray(input_mask)
    enc_mask = np.asarray(enc_mask)
    pr = {k: np.asarray(v, np.float32) for k, v in params.items()
          if k != "layers"}
    ly = {k: np.asarray(v, np.float32) for k, v in params["layers"].items()}

    # ---- shared (batch-independent) weight tensors
    shared = {}
    for src, dst in (("Wq_s", "Wq_s"), ("Wk_s", "Wk_s"), ("Wv_s", "Wv_s"),
                     ("Wq_e", "Wq_e"), ("Wk_e", "Wk_e"), ("Wv_e", "Wv_e")):
        w = ly[src].transpose(0, 2, 1, 3).reshape(L, D, 512)   # [L, d, h*dk]
        shared[dst] = np.ascontiguousarray(
            w.reshape(L, 4, P, 512).astype(bf16))
    for src, dst in (("bq_s", "bq_s"), ("bk_s", "bk_s"), ("bv_s", "bv_s"),
                     ("bq_e", "bq_e"), ("bk_e", "bk_e"), ("bv_e", "bv_e")):
        bb = ly[src].reshape(L, 512)
        shared[dst] = np.stack([_bias_cols(bb[l]) for l in range(L)])

    M_hv = np.empty((L, D, 512), np.float32)
    C_hv = np.empty((L, D, DC), np.float32)
    Pb = np.empty((L, DC), np.float32)
    for l in range(L):
        M_h = ly["v_W0"][l][:D] @ ly["v_W1"][l]
        M_v = ly["h_W0"][l][:D] @ ly["h_W1"][l]
        M_hv[l] = np.concatenate([M_h, M_v], axis=1)
        C_h = ly["v_W2"][l] @ ly["v_W3"][l] @ ly["f_W0"][l][:DC]
        C_v = ly["h_W2"][l] @ ly["h_W3"][l] @ ly["f_W0"][l][DC:]
        C_hv[l] = np.concatenate([C_h, C_v], axis=0)
        d_h = (ly["v_b2"][l] @ ly["v_W3"][l] + ly["v_b3"][l]) @ ly["f_W0"][l][:DC]
        d_v = (ly["h_b2"][l] @ ly["h_W3"][l] + ly["h_b3"][l]) @ ly["f_W0"][l][DC:]
        Pb[l] = d_h + d_v + ly["f_b0"][l]
    shared["M_hv"] = np.stack([_chunk_w(M_hv[l], 512) for l in range(L)])
    shared["C_hv"] = np.stack([_chunk_w(C_hv[l], DC) for l in range(L)])
    shared["Pb"] = np.stack([_bias_cols(Pb[l]) for l in range(L)])
    shared["fW1"] = np.stack([_chunk_w(ly["f_W1"][l], 512) for l in range(L)])
    shared["fb1"] = np.stack([_bias_cols(ly["f_b1"][l]) for l in range(L)])
    shared["fnnW0"] = np.stack([_chunk_w(ly["fnn_W0"][l], DF) for l in range(L)])
    shared["fnnb0"] = np.stack([_bias_cols(ly["fnn_b0"][l]) for l in range(L)])
    shared["fnnW1"] = np.stack([_chunk_w(ly["fnn_W1"][l], 512) for l in range(L)])
    shared["fnnb1"] = np.stack([_bias_cols(ly["fnn_b1"][l]) for l in range(L)])
    shared["lng"] = np.stack([_bias_cols(ly["ln_g"][l]) for l in range(L)])
    shared["lnb"] = np.stack([_bias_cols(ly["ln_b"][l]) for l in range(L)])
    shared["oW0"] = _chunk_w(pr["oW0"], 256)
    shared["ob0"] = _bias_cols(pr["ob0"])
    shared["oW1"] = _chunk_w(pr["oW1"], P)
    shared["ob1"] = _bias_cols(pr["ob1"])
    shared["oW2"] = np.ascontiguousarray(pr["oW2"].astype(bf16))
    shared["caus01"] = np.ascontiguousarray(
        np.triu(np.ones((P, P), np.float32)).astype(bf16))

    # ---- per-core tensors
    pos = _pos_table(150, D)[:T]
    emb = pr["emb"]
    in_maps = []
    for c in range(NCORES):
        bs = slice(c * BL, (c + 1) * BL)
        ids = input_batch[bs]
        x0 = emb[ids] * math.sqrt(D) + pos[None]           # [BL, T, D] f32
        x0T = x0.transpose(2, 0, 1).reshape(D, NT)
        m = dict(shared)
        m["x0T"] = np.ascontiguousarray(
            x0T.reshape(4, P, NT).astype(bf16))
        encT = encoder_out[bs].transpose(2, 0, 1).reshape(D, EC)
        m["encT"] = np.ascontiguousarray(
            encT.reshape(4, P, EC).astype(bf16))
        # folded per-batch context rows (with A-stage bias)
        c_hv = np.empty((L, BL, 512), np.float32)
        for l in range(L):
            c_h = (history_context[bs] @ ly["v_W0"][l][D:] @ ly["v_W1"][l]
                   + ly["v_b0"][l] @ ly["v_W1"][l] + ly["v_b1"][l])
            c_v = (visual_context[bs] @ ly["h_W0"][l][D:] @ ly["h_W1"][l]
                   + ly["h_b0"][l] @ ly["h_W1"][l] + ly["h_b1"][l])
            c_hv[l] = np.concatenate([c_h, c_v], axis=1)
        # [L, 128, 16]: col = mt*4 + b
        m["c_hv"] = np.ascontiguousarray(
            c_hv.reshape(L, BL, 4, P).transpose(0, 3, 2, 1).reshape(L, P, 16)
            .astype(np.float32))
        sb = np.where(input_mask[bs] == 0, NEG, 0.0).astype(np.float32)
        m["smask"] = np.ascontiguousarray(sb.T)            # [128, BL]
        eb = np.where(enc_mask[bs] == 0, NEG, 0.0).astype(np.float32)
        m["emask0"] = np.ascontiguousarray(eb[:, :P].T)
        m["emask1"] = np.ascontiguousarray(eb[:, P:].T)
        in_maps.append(m)
    return in_maps, pr["ob2"].astype(np.float32)


# ------------------------------------------------------------- entrypoint
_NC = None


def get_nc():
    global _NC
    if _NC is None:
        _NC = build_nc()
    return _NC


def kernel(input_batch, encoder_out, history_context, visual_context,
           input_mask, enc_mask, params):
    in_maps, ob2 = prep_inputs(input_batch, encoder_out, history_context,
                               visual_context, input_mask, enc_mask, params)
    nc = get_nc()
    res = run_bass_kernel_spmd(nc, in_maps, list(range(NCORES)))
    outs = [res.results[c]["out"].reshape(BL, T, V) for c in range(NCORES)]
    logits = np.concatenate(outs, axis=0).astype(np.float32)
    logits += ob2[None, None, :]
    return logits
